# revision 1
# baseline (speedup 1.0000x reference)
"""Trainium2 Bass kernel for the CoLL co-occurrence layer.

Math (per image):
    scale = 8/(max(x)-min(x)+1e-8)   (global over the whole batch)
    u     = (x - xmin)*scale ;  idx = clip(floor(u), 0, 7)
    y(p)  = sum_q w[q] * x(p+q) * co[idx_p, idx_q]   over 3x3 neighborhoods q

Reformulation (staircase basis, select form):
    g_j(p) = 1[u(p) >= j]                j = 1..7   (g_0 == 1)
    m_j    = x * g_j                     (m_0 = x)
    n_i    = sum_j A[i,j] m_j            A = column-diffs of co  -> n_i = x * co[i, idx]
    V_i    = conv3x3(n_i, w)             (SAME, zero pad)
    y(p)   = V_{idx_p}(p)                via a chain of predicated copies on g_i

Mapping (one image per NeuronCore, batch 8 over 8 cores):
  - binning (u, g) is exact fp32, matching the reference op-for-op; everything
    heavy (masked fields, mix, conv) runs in bf16 (tolerance is 2e-2; fp32
    matmuls are 4x slower per column on the PE).
  - layout [h=128 partitions, (w,c)=8192 free]; conv along h via banded-matrix
    matmuls on PE, conv along w via +-C free-dim reads of a zero-padded tile.
  - the 8x8 mix runs on PE with fields packed on partitions by stride-8 row
    groups: group a = rows {a, a+8, ..., a+120}; P[l*8+j, a, :] = m_j(a+8l, :),
    weight kron(I16, A^T), one matmul per group per PSUM-bank span. The
    stride-8 grouping makes both pack (mP -> P) and unpack (npack -> nnat)
    plain SBUF-SBUF DMAs whose destinations span all 128 partitions, and the
    unpacked rows land back in natural order (p = 8l + a = h).
  - PSUM: 2x double-buffered 2-bank mix tiles + 1-bank tail + 3 conv banks
    (2-field conv groups) = 8 banks; PSUM evacuation and V_0 copies on ACT;
    the select chain (copy_predicated) reads conv PSUM directly on DVE.
  - global min/max + 2-float AllReduce(max) of (-min, max) for the scale.
"""

from contextlib import ExitStack

import numpy as np

import concourse.bass as bass
import concourse.tile as tile
from concourse import mybir
from concourse.tile_rust import add_dep_helper

F32 = mybir.dt.float32
BF16 = mybir.dt.bfloat16
U16 = mybir.dt.uint16
AX = mybir.AxisListType
OP = mybir.AluOpType

N, H, W, C = 8, 128, 128, 64
NB = 8
N_CORES = 8
Fd = W * C            # 8192
FC = 1024             # chunk width (output cols per chunk)
EXT = FC + 2 * C      # 1152: chunk + 64-halo each side
NCH = Fd // FC        # 8 chunks


def build_tables(co, w):
    """Host-side weight-matrix construction from the tiny co/w inputs.

    mixW [128,128] bf16: kron(I16, A^T) with A = column-diffs of co, so that
      out[lo*8+i, c] = sum_j A[i,j] * in[li*8+j, c]  (li==lo).
    band [3,128,128] bf16: band[dw, hi, ho] = w[ho-hi+1, dw] (|ho-hi|<=1).
    """
    co = np.asarray(co, np.float32)
    w = np.asarray(w, np.float32)
    A = co - np.concatenate([np.zeros((NB, 1), np.float32), co[:, :-1]], axis=1)
    mixW = np.kron(np.eye(16, dtype=np.float32), A.T.copy())      # [128,128]
    band = np.zeros((3, 128, 128), np.float32)  # [dw, h_in, h_out]
    for dw in range(3):
        for ho in range(128):
            for dh in range(3):
                hi = ho + dh - 1
                if 0 <= hi < 128:
                    band[dw, hi, ho] = w[dh, dw]
    return {"mixW": mixW, "band": band}


def build_bass(n_cores=N_CORES, reps=1, FC_unused=None):
    """Per-core Bass module; every core runs the same program on its own image
    (collective min/max when n_cores > 1). reps>1 wraps the main pipeline in a
    For_i for wall-clock HW timing."""
    from concourse.bacc import Bacc
    nc = Bacc()
    x_d = nc.declare_dram_parameter("x", [H, Fd], F32, isOutput=False)
    mixW_d = nc.declare_dram_parameter("mixW", [128, 128], F32, isOutput=False)
    band_d = nc.declare_dram_parameter("band", [3, 128, 128], F32, isOutput=False)
    y_d = nc.declare_dram_parameter("y", [H, Fd], F32, isOutput=True)
    cc_in = nc.dram_tensor("cc_in", [2], F32)
    if n_cores > 1:
        cc_out = nc.dram_tensor("cc_out", [2], F32, addr_space="Shared")

    with tile.TileContext(nc) as tc, ExitStack() as ctx:
        consts = ctx.enter_context(tc.tile_pool(name="consts", bufs=1))
        upool = ctx.enter_context(tc.tile_pool(name="upool", bufs=1))
        gpool = ctx.enter_context(tc.tile_pool(name="gpool", bufs=2))
        mpool = ctx.enter_context(tc.tile_pool(name="mpool", bufs=2))
        ppool = ctx.enter_context(tc.tile_pool(name="ppool", bufs=1))
        mixps = ctx.enter_context(tc.tile_pool(name="mixps", bufs=2, space="PSUM"))
        mixpst = ctx.enter_context(tc.tile_pool(name="mixpst", bufs=1, space="PSUM"))
        npackp = ctx.enter_context(tc.tile_pool(name="npackp", bufs=1))
        nnatp = ctx.enter_context(tc.tile_pool(name="nnatp", bufs=2))
        convps = ctx.enter_context(tc.tile_pool(name="convps", bufs=3, space="PSUM"))
        ypool = ctx.enter_context(tc.tile_pool(name="ypool", bufs=2))

        # ---- constants: weights + bf16 copy of x, resident in SBUF ----
        mixWf = consts.tile([128, 128], F32)
        nc.sync.dma_start(out=mixWf, in_=mixW_d[:, :])
        mixW = consts.tile([128, 128], BF16)
        nc.vector.tensor_copy(mixW, mixWf)
        bandf = consts.tile([128, 3, 128], F32)
        nc.sync.dma_start(out=bandf, in_=band_d[:, :, :].rearrange("d i o -> i d o"))
        band = consts.tile([128, 3, 128], BF16)
        nc.vector.tensor_copy(band, bandf)

        # zero-padded images resident in SBUF: fp32 (exact binning) + bf16
        xpad = consts.tile([128, Fd + 2 * C], BF16)
        nc.gpsimd.memset(xpad[:, 0:C], 0.0)
        nc.gpsimd.memset(xpad[:, C + Fd:], 0.0)
        xfp = consts.tile([128, Fd + 2 * C], F32)
        nc.gpsimd.memset(xfp[:, 0:C], 0.0)
        nc.gpsimd.memset(xfp[:, C + Fd:], 0.0)
        nc.sync.dma_start(out=xfp[:, C:C + Fd], in_=x_d[:, :])
        for ci in range(NCH):
            nc.scalar.copy(xpad[:, C + ci * FC:C + (ci + 1) * FC],
                           xfp[:, C + ci * FC:C + (ci + 1) * FC])

        # ---- global min/max -> scale (exact fp32, matches reference) ----
        mn = consts.tile([128, 1], F32)
        mx = consts.tile([128, 1], F32)
        nc.vector.tensor_reduce(mn, xfp[:, C:C + Fd], axis=AX.X, op=OP.min)
        nc.vector.tensor_reduce(mx, xfp[:, C:C + Fd], axis=AX.X, op=OP.max)
        tmp = consts.tile([128, 2], F32)
        nc.vector.tensor_scalar_mul(tmp[:, 0:1], mn, -1.0)   # (-min, max)
        nc.vector.tensor_copy(tmp[:, 1:2], mx)
        red = consts.tile([1, 2], F32)
        nc.gpsimd.tensor_reduce(red, tmp, axis=AX.C, op=OP.max)
        pair = consts.tile([128, 2], F32)
        dma_in = nc.sync.dma_start(out=cc_in[:], in_=red)
        src = cc_in
        prev = dma_in
        if n_cores > 1:
            cc = nc.gpsimd.collective_compute(
                "AllReduce", OP.max,
                replica_groups=[list(range(n_cores))],
                ins=[cc_in.ap().opt()], outs=[cc_out.ap().opt()],
            )
            add_dep_helper(cc.ins, dma_in.ins, True, "cc waits dram write")
            src = cc_out
            prev = cc
        bcast = bass.AP(tensor=src.ap().tensor, offset=0, ap=[[0, 128], [1, 2]])
        dma_back = nc.sync.dma_start(out=pair[:, :], in_=bcast)
        add_dep_helper(dma_back.ins, prev.ins, True, "bcast waits dram ready")
        negxmin = pair[:, 0:1]
        gmax = pair[:, 1:2]

        rng = consts.tile([128, 1], F32)
        nc.vector.tensor_tensor(rng, gmax, negxmin, op=OP.add)   # xmax - xmin
        dd = consts.tile([128, 1], F32)
        nc.vector.tensor_scalar_add(dd, rng, float(np.float32(1e-8)))
        # scale = 8/d as 8*(1/d): exact wrt fl(8/d) since *8 is a pow2 scale
        recip = consts.tile([128, 1], F32)
        nc.vector.reciprocal(recip, dd)
        scale = consts.tile([128, 1], F32)
        nc.vector.tensor_scalar_mul(scale, recip, 8.0)

        # ---- main streamed pipeline over free-dim chunks ----
        def chunk_pipeline(ci):
            cs = ci * FC  # data cols [cs-C, cs+FC+C) = xpad cols [cs, cs+EXT)
            u = upool.tile([128, EXT], F32, tag="u")
            nc.vector.tensor_scalar(u, xfp[:, cs:cs + EXT], negxmin, scale,
                                    op0=OP.add, op1=OP.mult)
            g = gpool.tile([128, 7, EXT], BF16, tag="g")
            for j in range(7):
                nc.vector.tensor_scalar(g[:, j, :], u, float(j + 1), None,
                                        op0=OP.is_ge)

            # masked fields, j-minor layout: mP[:, 0, :] = x, mP[:, j, :] = x*g_j
            mP = mpool.tile([128, NB, EXT], BF16, tag="mP")
            nc.vector.tensor_copy(mP[:, 0, :], xpad[:, cs:cs + EXT])
            for j in range(1, NB):
                nc.vector.tensor_tensor(mP[:, j, :], xpad[:, cs:cs + EXT],
                                        g[:, j - 1, :], op=OP.mult)

            # pack per stride-8 row group a (rows a, a+8, ..., a+120):
            # P[l*8+j, a, :] = mP[a+8l, j, :] — dest spans all 128 partitions,
            # src spans 8 DMA port-clusters (stride-8 partitions)
            P = ppool.tile([128, 8, EXT], BF16, tag="P")
            for a in range(8):
                nc.sync.dma_start(out=P[:, a, :], in_=mP[a:128:8, :, :])

            # mix: n[lo*8+i] = sum_j A[i,j] m_j  (kron(I16, A^T) stationary)
            npack = npackp.tile([128, 8, EXT], BF16, tag="npack")
            for s in range(8):
                pm = mixps.tile([128, 1024], F32, tag="pm")
                pmt = mixpst.tile([128, 512], F32, tag="pmt")
                nc.tensor.matmul(pm[:, 0:512], mixW, P[:, s, 0:512],
                                 start=True, stop=True)
                nc.tensor.matmul(pm[:, 512:1024], mixW, P[:, s, 512:1024],
                                 start=True, stop=True)
                nc.tensor.matmul(pmt[:, 0:128], mixW, P[:, s, 1024:EXT],
                                 start=True, stop=True)
                # evacuate PSUM -> SBUF bf16 on ACT
                nc.scalar.copy(npack[:, s, 0:1024], pm)
                nc.scalar.copy(npack[:, s, 1024:EXT], pmt[:, 0:128])

            # unpack per field i: nnat[8l+a, i, :] = npack[l*8+i, a, :] — the
            # stride-8 grouping makes rows land back in natural order, and the
            # dest spans all 128 partitions (cheap DMA)
            nnat = nnatp.tile([128, NB, EXT], BF16, tag="nnat")
            for i in range(NB):
                nc.sync.dma_start(out=nnat[:, i, :], in_=npack[i:128:8, :, :])

            # conv + select, per 512-col span
            y_t = ypool.tile([128, FC], F32, tag="y")
            for sp in range(0, FC, 512):
                for half in range(4):
                    vts = []
                    for k in range(2):
                        vt = convps.tile([128, 512], F32, tag="vt")
                        vts.append(vt)
                    for dw in range(3):
                        for k in range(2):
                            i = 2 * half + k
                            nc.tensor.matmul(
                                vts[k], band[:, dw, :],
                                nnat[:, i, sp + dw * C:sp + dw * C + 512],
                                start=(dw == 0), stop=(dw == 2))
                    for k in range(2):
                        i = 2 * half + k
                        if i == 0:
                            # V_0 baseline copy on ACT (frees DVE for preds)
                            nc.scalar.copy(y_t[:, sp:sp + 512], vts[k])
                        else:
                            mask = g[:, i - 1, C + sp:C + sp + 512].bitcast(U16)
                            nc.vector.copy_predicated(y_t[:, sp:sp + 512],
                                                      mask, vts[k])
            nc.sync.dma_start(out=y_d[:, cs:cs + FC], in_=y_t)

        if reps == 1:
            for ci in range(NCH):
                chunk_pipeline(ci)
        else:
            with tc.For_i(0, reps, 1):
                for ci in range(NCH):
                    chunk_pipeline(ci)
    nc.finalize()
    return nc


_CACHE = {}


def _run(x, co_matrix, w_spatial, trace=False):
    x = np.ascontiguousarray(np.asarray(x, np.float32))
    tables = build_tables(co_matrix, w_spatial)
    n, h, w_, c = x.shape
    assert (n, h, w_, c) == (N, H, W, C), (n, h, w_, c)

    from concourse.bass_utils import run_bass_kernel_spmd

    key = "full"
    if key not in _CACHE:
        _CACHE[key] = build_bass(n_cores=N_CORES)
    nc = _CACHE[key]

    in_maps = []
    for core in range(N_CORES):
        in_maps.append({"x": x[core].reshape(H, W * C), **tables})
    res = run_bass_kernel_spmd(nc, in_maps, core_ids=list(range(N_CORES)),
                               trace=trace)
    out = np.stack([res.results[i]["y"].reshape(H, W, C) for i in range(N_CORES)], 0)
    return out, res


def kernel(x, co_matrix, w_spatial):
    return _run(x, co_matrix, w_spatial)[0]


def run_traced(x, co_matrix, w_spatial):
    _, res = _run(x, co_matrix, w_spatial, trace=True)
    return res.exec_time_ns

